# revision 3
# baseline (speedup 1.0000x reference)
"""Causal self-attention Trainium2 Bass kernel (v2).

Problem: B=16, T=2048, D=128, H=4 (head dim 32), fp32 in/out.
  qkv = x @ w_attn ; per-head scores = q k^T / sqrt(32), causal
  y = softmax(scores) @ v ; out = y @ w_proj

Sharding: data-parallel over batch, 2 batches per NeuronCore x 8 cores
(SPMD, no collectives).

v2 design (vs the 428us baseline):
  - ScalarE (exp) is the roofline engine (~150us serial of exp work per
    core); everything else is structured to overlap under it.
  - denominator trick: V' per head = [V_h | ones32 | zeros] so attn@V
    puts softmax sums on PSUM rows 32:64 of the same tile as y (rows
    0:32); normalization is one partition-shifted DVE reciprocal + two
    muls. No expander matmul, no sums staging copies.
  - fp16 operands everywhere on the PE (QKV projection, scores, attn@V,
    out projection); fp32 only for the exact recompute of query rows
    0:128 (they attend over few keys, fp16 noise is not averaged away).
  - diagonal-band trimming: scores matmul, exp, and attn@V all stream
    only columns >= r*128 of a diag chunk; nothing reads the masked
    region so no et zero-fill is needed.
  - out projection contracts K=64 (pair's 64 live ysb rows): no zero
    padding to maintain, pooled ysb tiles.
  - single pool scope; emission interleaves batch-1 phase A into
    batch-0 phase B so QKV/transpose PE+DVE work fills gaps under ACT.

Hardware traps (TRN2, carried over from baseline):
  - M<128 col-tiled matmuls (tile_position=(0,64)) crash / corrupt;
    all attn@V matmuls use M=128, out-proj uses K=64 at (0,0).
  - accumulating K=32 matmuls at different row positions into one PSUM
    bank crash; scores quads use start/stop=True per chunk.
  - engine partition offsets must be 32-aligned; 32-step cross-operand
    partition shifts are legal on DVE (used by reciprocal/mul).
"""

import os
import sys

import numpy as np

for _p in ("/opt/trn_rl_repo", "/root/.axon_site/_ro/trn_rl_repo"):
    if os.path.isdir(_p) and _p not in sys.path:
        sys.path.insert(0, _p)

import concourse.bass as bass
import concourse.bacc as bacc
import concourse.mybir as mybir
import concourse.tile as tile
from concourse.bass_utils import run_bass_kernel_spmd

F32 = mybir.dt.float32
F16 = mybir.dt.float16
P = 128
NEG = -1.0e9

B, T, D, H = 16, 2048, 128, 4
DH = D // H  # 32
N_CORES = 8
BPC = B // N_CORES  # batches per core


def build_attention_nc(bpc=BPC, t=T, qg=512, loop_n=0, attn_dt="f16",
                       recompute=True, interleave=True, quad_bufs=2,
                       exp_bufs=6):
    assert t % qg == 0 and qg % P == 0 and t % P == 0
    nqg = t // qg      # q groups
    nkc = t // P       # 128-wide k chunks
    cpq = qg // P      # k chunks per q group
    scale = 1.0 / float(np.sqrt(DH))
    FA = {"f16": F16, "f32": F32}[attn_dt]

    nc = bacc.Bacc("TRN2", target_bir_lowering=False, debug=False)
    x_d = nc.dram_tensor("x", [bpc, t, D], F32, kind="ExternalInput")
    wa_d = nc.dram_tensor("w_attn", [D, 3 * D], F32, kind="ExternalInput")
    wp_d = nc.dram_tensor("w_proj", [D, D], F32, kind="ExternalInput")
    out_d = nc.dram_tensor("out", [bpc, t, D], F32, kind="ExternalOutput")

    with tile.TileContext(nc) as tc:
        with (
            tc.tile_pool(name="resident", bufs=1) as res,
            tc.tile_pool(name="xin", bufs=8) as xin_pool,
            tc.tile_pool(name="ps", bufs=quad_bufs, space="PSUM") as ps_pool,
            tc.tile_pool(name="ypsum", bufs=2, space="PSUM") as y_pool,
            tc.tile_pool(name="expt", bufs=exp_bufs) as exp_pool,
            tc.tile_pool(name="recp", bufs=3) as rec_pool,
            tc.tile_pool(name="ysb", bufs=4) as ysb_pool,
            tc.tile_pool(name="outsb", bufs=6) as out_pool,
            tc.tile_pool(name="r32", bufs=2) as r_pool,
        ):
            # ---- resident constants ----
            wa32 = res.tile([D, 3 * D], F32, name="wa32", tag="wa32")
            nc.sync.dma_start(wa32[:], wa_d[:])
            wa16 = res.tile([D, 3 * D], FA, name="wa16", tag="wa16")
            nc.vector.tensor_copy(wa16[:], wa32[:])

            # w_proj: pair pi's tile holds head 2pi rows at 0:32 and head
            # 2pi+1 rows at 32:64 (out-proj contracts K=64)
            wp32 = res.tile([D, D], F32, name="wp32", tag="wp32")
            nc.sync.dma_start(wp32[:], wp_d[:])
            wpP = [res.tile([2 * DH, D], FA, name=f"wp{pi}", tag=f"wp{pi}")
                   for pi in range(2)]
            for pi in range(2):
                nc.vector.tensor_copy(wpP[pi][:], wp32[2 * DH * pi:
                                                       2 * DH * (pi + 1), :])

            ident = res.tile([P, P], F32, name="ident", tag="ident")
            nc.gpsimd.memset(ident[:], 0.0)
            nc.gpsimd.affine_select(
                out=ident[:], in_=ident[:],
                compare_op=mybir.AluOpType.not_equal, fill=1.0,
                base=0, pattern=[[-1, P]], channel_multiplier=1,
            )

            # causal additive mask for the diagonal 128x128 block:
            # cmask[k, c] = 0 if c >= k else NEG
            cmask = res.tile([P, P], F32, name="cmask", tag="cmask")
            nc.gpsimd.memset(cmask[:], 0.0)
            nc.gpsimd.affine_select(
                out=cmask[:], in_=cmask[:],
                compare_op=mybir.AluOpType.is_ge, fill=NEG,
                base=0, pattern=[[1, P]], channel_multiplier=-1,
            )

            # ---- per-batch resident activations ----
            xT = [res.tile([P, t], FA, name=f"xT{b}", tag=f"xT{b}")
                  for b in range(bpc)]
            qT = [res.tile([P, t], FA, name=f"qT{b}", tag=f"qT{b}")
                  for b in range(bpc)]
            kT = [res.tile([P, t], FA, name=f"kT{b}", tag=f"kT{b}")
                  for b in range(bpc)]
            # V' per (b, kc, h): [128 kpos, 128] = [V_h | ones32 | zeros];
            # ones block at cols 32:64 -> attn@V emits softmax sums on
            # psum rows 32:64 (same tile as y rows 0:32)
            vP = [res.tile([P, nkc, H, P], FA, name=f"vp{b}", tag=f"vp{b}")
                  for b in range(bpc)]
            for b in range(bpc):
                nc.gpsimd.memset(vP[b][:], 0.0)
                nc.gpsimd.memset(vP[b][:, :, :, DH:2 * DH], 1.0)

            # fp32 shadows for the exact recompute of query rows 0:128
            if recompute:
                xT32 = [res.tile([P, P], F32, name=f"xT32_{b}",
                                 tag=f"xT32_{b}") for b in range(bpc)]
                qT32 = [res.tile([P, P], F32, name=f"qT32_{b}",
                                 tag=f"qT32_{b}") for b in range(bpc)]
                kT32 = [res.tile([P, P], F32, name=f"kT32_{b}",
                                 tag=f"kT32_{b}") for b in range(bpc)]
                vP32 = [res.tile([P, H, P], F32, name=f"vP32_{b}",
                                 tag=f"vP32_{b}") for b in range(bpc)]
                for b in range(bpc):
                    nc.gpsimd.memset(vP32[b][:], 0.0)
                    nc.gpsimd.memset(vP32[b][:, :, DH:2 * DH], 1.0)

            import contextlib
            loop_cm = (tc.For_i(0, loop_n, 1) if loop_n
                       else contextlib.nullcontext())
            with loop_cm:

                # ============ phase A (emitted in pieces) ============
                def emit_A_transpose(b, j):
                    """transpose chunks 4j..4j+3 of x into xT[b]."""
                    for kc in range(4 * j, 4 * j + 4):
                        xi = xin_pool.tile([P, D], F32, name="xin",
                                           tag="xin")
                        nc.sync.dma_start(xi[:],
                                          x_d[b, kc * P:(kc + 1) * P, :])
                        pst = ps_pool.tile([P, 2, qg], F32, name="pst",
                                           tag="ps")
                        nc.tensor.transpose(pst[:, 0, 0:P], xi[:], ident[:])
                        nc.vector.tensor_copy(
                            xT[b][:, kc * P:(kc + 1) * P], pst[:, 0, 0:P])
                        if recompute and kc == 0:
                            nc.vector.tensor_copy(xT32[b][:],
                                                  pst[:, 0, 0:P])

                def emit_A_qkv(b, j):
                    """QKV^T for token block j (512 wide)."""
                    sl = slice(j * qg, (j + 1) * qg)
                    for wofs, dst in ((0, qT[b]), (D, kT[b])):
                        pq = ps_pool.tile([P, 2, qg], F32, name="pq",
                                          tag="ps")
                        nc.tensor.matmul(
                            pq[:, 0, :], wa16[:, wofs:wofs + D],
                            xT[b][:, sl], start=True, stop=True)
                        nc.vector.tensor_copy(dst[:, sl], pq[:, 0, :])
                    for kc in range(4 * j, 4 * j + 4):
                        pv = ps_pool.tile([P, 2, qg], F32, name="pv",
                                          tag="ps")
                        nc.tensor.matmul(
                            pv[:, 0, 0:P], xT[b][:, kc * P:(kc + 1) * P],
                            wa16[:, 2 * D:3 * D], start=True, stop=True)
                        nc.vector.tensor_copy(
                            vP[b][:, kc, :, 0:DH],
                            pv[:, 0, 0:P].rearrange("p (h d) -> p h d",
                                                    h=H))
                        if recompute and kc == 0:
                            nc.vector.tensor_copy(
                                vP32[b][:, :, 0:DH],
                                pv[:, 0, 0:P].rearrange("p (h d) -> p h d",
                                                        h=H))

                def emit_A_recompute_qk(b):
                    """exact fp32 q/k for the first 128 tokens."""
                    for wofs, dst in ((0, qT32[b]), (D, kT32[b])):
                        pq = ps_pool.tile([P, 2, qg], F32, name="pq32",
                                          tag="ps")
                        nc.tensor.matmul(
                            pq[:, 0, 0:P], wa32[:, wofs:wofs + D],
                            xT32[b][:], start=True, stop=True)
                        nc.vector.tensor_copy(dst[:], pq[:, 0, 0:P])

                # ============ phase B: one (q-group, head-pair) ============
                def emit_B_group(b, j, pi):
                    qsl = slice(j * qg, (j + 1) * qg)
                    kmax = cpq * (j + 1) - 1
                    y2 = y_pool.tile([P, 2, qg], F32, name="y", tag="y")
                    for kc in range(kmax + 1):
                        ksl = slice(kc * P, (kc + 1) * P)
                        r = kc - cpq * j  # diag band index (>=0: on band)
                        c0 = max(r, 0) * P
                        quad = ps_pool.tile([P, 2, qg], F32, name="quad",
                                            tag="ps")
                        for ci in range(2):
                            h = 2 * pi + ci
                            hp = slice(32 * h, 32 * h + 32)
                            nc.tensor.matmul(
                                quad[:, ci, c0:], kT[b][hp, ksl],
                                qT[b][hp, j * qg + c0:(j + 1) * qg],
                                start=True, stop=True,
                                tile_position=(32 * h, 0))
                        et = exp_pool.tile([P, 2, qg], FA, name="et",
                                           tag="et")
                        if r >= 0:
                            blk = slice(c0, c0 + P)
                            nc.vector.tensor_tensor(
                                quad[:, :, blk], quad[:, :, blk],
                                cmask[:, None, :].to_broadcast((P, 2, P)),
                                mybir.AluOpType.add)
                        nc.scalar.activation(
                            et[:, :, c0:], quad[:, :, c0:],
                            mybir.ActivationFunctionType.Exp, scale=scale)
                        st = kc == 0
                        sp = kc == kmax
                        for ci in range(2):
                            h = 2 * pi + ci
                            nc.tensor.matmul(
                                y2[:, ci, c0:], vP[b][:, kc, h, :],
                                et[:, ci, c0:],
                                start=st, stop=sp, skip_group_check=True)
                    if recompute and j == 0:
                        # exact fp32 recompute of query rows 0:128;
                        # overwrites y columns 0:128 (start=True)
                        q32 = ps_pool.tile([P, 2, qg], F32, name="q32",
                                           tag="ps")
                        for ci in range(2):
                            h = 2 * pi + ci
                            hp = slice(32 * h, 32 * h + 32)
                            nc.tensor.matmul(
                                q32[:, ci, 0:P], kT32[b][hp, :],
                                qT32[b][hp, :], start=True, stop=True,
                                tile_position=(32 * h, 0))
                        nc.vector.tensor_tensor(
                            q32[:, :, 0:P], q32[:, :, 0:P],
                            cmask[:, None, :].to_broadcast((P, 2, P)),
                            mybir.AluOpType.add)
                        et32 = r_pool.tile([P, 2, P], F32, name="et32",
                                           tag="et32")
                        nc.scalar.activation(
                            et32[:], q32[:, :, 0:P],
                            mybir.ActivationFunctionType.Exp, scale=scale)
                        for ci in range(2):
                            h = 2 * pi + ci
                            nc.tensor.matmul(
                                y2[:, ci, 0:P], vP32[b][:, h, :],
                                et32[:, ci, :],
                                start=True, stop=True,
                                skip_group_check=True)
                    # ---- normalize: y rows 0:32, sums rows 32:64 ----
                    rec = rec_pool.tile([DH, 2, qg], F32, name="rec",
                                        tag="rec")
                    nc.vector.reciprocal(rec[:], y2[DH:2 * DH, :, :])
                    ysb = ysb_pool.tile([2 * DH, qg], FA, name="ysb",
                                        tag="ysb")
                    nc.vector.tensor_mul(ysb[0:DH, :], y2[0:DH, 0, :],
                                         rec[:, 0, :])
                    nc.vector.tensor_mul(ysb[DH:2 * DH, :], y2[0:DH, 1, :],
                                         rec[:, 1, :])
                    return ysb

                def emit_B_proj(b, j, ysbs):
                    for tch in range(cpq):
                        t0 = j * qg + tch * P
                        csl = slice(tch * P, (tch + 1) * P)
                        po = ps_pool.tile([P, 2, qg], F32, name="proj",
                                          tag="ps")
                        for pi in range(2):
                            nc.tensor.matmul(
                                po[:, 0, 0:D], ysbs[pi][:, csl], wpP[pi][:],
                                start=(pi == 0), stop=(pi == 1),
                                skip_group_check=True)
                        ob = out_pool.tile([P, D], F32, name="ob", tag="ob")
                        nc.vector.tensor_copy(ob[:], po[:, 0, 0:D])
                        nc.sync.dma_start(out_d[b, t0:t0 + P, :], ob[:])

                def emit_A(b):
                    for j in range(nqg):
                        emit_A_transpose(b, j)
                        emit_A_qkv(b, j)
                    if recompute:
                        emit_A_recompute_qk(b)

                def emit_B(b):
                    for j in range(nqg):
                        ysbs = [emit_B_group(b, j, pi) for pi in range(2)]
                        emit_B_proj(b, j, ysbs)

                if interleave and bpc == 2:
                    # A(0); then B(0) groups with A(1) pieces woven in so
                    # batch-1 QKV fills PE/DVE gaps under batch-0's
                    # ACT-bound attention; then B(1).
                    emit_A(0)
                    a1 = []
                    for j in range(nqg):
                        a1.append(lambda j=j: emit_A_transpose(1, j))
                        a1.append(lambda j=j: emit_A_qkv(1, j))
                    if recompute:
                        a1.append(lambda: emit_A_recompute_qk(1))
                    ai = 0
                    for j in range(nqg):
                        ysbs = []
                        for pi in range(2):
                            ysbs.append(emit_B_group(0, j, pi))
                            if ai < len(a1):
                                a1[ai]()
                                ai += 1
                        emit_B_proj(0, j, ysbs)
                    while ai < len(a1):
                        a1[ai]()
                        ai += 1
                    emit_B(1)
                else:
                    for b in range(bpc):
                        emit_A(b)
                    for b in range(bpc):
                        emit_B(b)
    nc.compile()
    return nc


_NC_CACHE = {}

CONFIG = {"attn_dt": "f16", "recompute": True, "interleave": True,
          "quad_bufs": 2, "exp_bufs": 6}


def _get_nc(bpc=BPC, t=T, loop_n=0):
    key = (bpc, t, loop_n)
    if key not in _NC_CACHE:
        _NC_CACHE[key] = build_attention_nc(bpc=bpc, t=t, loop_n=loop_n,
                                            **CONFIG)
    return _NC_CACHE[key]


def _run(x, w_attn, w_proj, **spmd_kwargs):
    x = np.ascontiguousarray(np.asarray(x), dtype=np.float32)
    w_attn = np.ascontiguousarray(np.asarray(w_attn), dtype=np.float32)
    w_proj = np.ascontiguousarray(np.asarray(w_proj), dtype=np.float32)
    assert x.shape == (B, T, D), x.shape

    nc = _get_nc()
    in_maps = [
        {"x": x[c * BPC:(c + 1) * BPC], "w_attn": w_attn, "w_proj": w_proj}
        for c in range(N_CORES)
    ]
    res = run_bass_kernel_spmd(nc, in_maps, list(range(N_CORES)),
                               **spmd_kwargs)
    out = np.concatenate([res.results[c]["out"] for c in range(N_CORES)],
                         axis=0)
    return out.astype(np.float32), res


def kernel(x, w_attn, w_proj):
    out, _ = _run(x, w_attn, w_proj)
    return out


if __name__ == "__main__":
    nc = build_attention_nc()
    print("built ok")


# revision 4
# speedup vs baseline: 1.1985x; 1.1985x over previous
"""Causal self-attention Trainium2 Bass kernel (v2).

Problem: B=16, T=2048, D=128, H=4 (head dim 32), fp32 in/out.
  qkv = x @ w_attn ; per-head scores = q k^T / sqrt(32), causal
  y = softmax(scores) @ v ; out = y @ w_proj

Sharding: data-parallel over batch, 2 batches per NeuronCore x 8 cores
(SPMD, no collectives).

v2 design (vs the 428us baseline):
  - ScalarE (exp) is the roofline engine (~150us serial of exp work per
    core); everything else is structured to overlap under it.
  - denominator trick: V' per head = [V_h | ones32 | zeros] so attn@V
    puts softmax sums on PSUM rows 32:64 of the same tile as y (rows
    0:32); normalization is one partition-shifted DVE reciprocal + two
    muls. No expander matmul, no sums staging copies.
  - fp16 operands everywhere on the PE (QKV projection, scores, attn@V,
    out projection); fp32 only for the exact recompute of query rows
    0:128 (they attend over few keys, fp16 noise is not averaged away).
  - diagonal-band trimming: scores matmul, exp, and attn@V all stream
    only columns >= r*128 of a diag chunk; nothing reads the masked
    region so no et zero-fill is needed.
  - out projection contracts K=64 (pair's 64 live ysb rows): no zero
    padding to maintain, pooled ysb tiles.
  - single pool scope; emission interleaves batch-1 phase A into
    batch-0 phase B so QKV/transpose PE+DVE work fills gaps under ACT.

Hardware traps (TRN2, carried over from baseline):
  - M<128 col-tiled matmuls (tile_position=(0,64)) crash / corrupt;
    all attn@V matmuls use M=128, out-proj uses K=64 at (0,0).
  - accumulating K=32 matmuls at different row positions into one PSUM
    bank crash; scores quads use start/stop=True per chunk.
  - engine partition offsets must be 32-aligned; 32-step cross-operand
    partition shifts are legal on DVE (used by reciprocal/mul).
"""

import os
import sys

import numpy as np

for _p in ("/opt/trn_rl_repo", "/root/.axon_site/_ro/trn_rl_repo"):
    if os.path.isdir(_p) and _p not in sys.path:
        sys.path.insert(0, _p)

import concourse.bass as bass
import concourse.bacc as bacc
import concourse.mybir as mybir
import concourse.tile as tile
from concourse.bass_utils import run_bass_kernel_spmd

F32 = mybir.dt.float32
F16 = mybir.dt.float16
P = 128
NEG = -1.0e9

B, T, D, H = 16, 2048, 128, 4
DH = D // H  # 32
N_CORES = 8
BPC = B // N_CORES  # batches per core


def build_attention_nc(bpc=BPC, t=T, qg=512, loop_n=0, attn_dt="f16",
                       recompute=True, interleave=True, quad_bufs=2,
                       exp_bufs=6):
    assert t % qg == 0 and qg % P == 0 and t % P == 0
    nqg = t // qg      # q groups
    nkc = t // P       # 128-wide k chunks
    cpq = qg // P      # k chunks per q group
    scale = 1.0 / float(np.sqrt(DH))
    FA = {"f16": F16, "f32": F32}[attn_dt]

    nc = bacc.Bacc("TRN2", target_bir_lowering=False, debug=False)
    x_d = nc.dram_tensor("x", [bpc, t, D], F32, kind="ExternalInput")
    wa_d = nc.dram_tensor("w_attn", [D, 3 * D], F32, kind="ExternalInput")
    wp_d = nc.dram_tensor("w_proj", [D, D], F32, kind="ExternalInput")
    out_d = nc.dram_tensor("out", [bpc, t, D], F32, kind="ExternalOutput")

    with tile.TileContext(nc) as tc:
        with (
            tc.tile_pool(name="resident", bufs=1) as res,
            tc.tile_pool(name="xin", bufs=8) as xin_pool,
            tc.tile_pool(name="ps", bufs=quad_bufs, space="PSUM") as ps_pool,
            tc.tile_pool(name="ypsum", bufs=2, space="PSUM") as y_pool,
            tc.tile_pool(name="expt", bufs=exp_bufs) as exp_pool,
            tc.tile_pool(name="recp", bufs=3) as rec_pool,
            tc.tile_pool(name="ysb", bufs=4) as ysb_pool,
            tc.tile_pool(name="outsb", bufs=6) as out_pool,
            tc.tile_pool(name="r32", bufs=2) as r_pool,
        ):
            # ---- resident constants ----
            wa32 = res.tile([D, 3 * D], F32, name="wa32", tag="wa32")
            nc.sync.dma_start(wa32[:], wa_d[:])
            wa16 = res.tile([D, 3 * D], FA, name="wa16", tag="wa16")
            nc.vector.tensor_copy(wa16[:], wa32[:])

            # w_proj: pair pi's tile holds head 2pi rows at 0:32 and head
            # 2pi+1 rows at 32:64 (out-proj contracts K=64)
            wp32 = res.tile([D, D], F32, name="wp32", tag="wp32")
            nc.sync.dma_start(wp32[:], wp_d[:])
            wpP = [res.tile([2 * DH, D], FA, name=f"wp{pi}", tag=f"wp{pi}")
                   for pi in range(2)]
            for pi in range(2):
                nc.vector.tensor_copy(wpP[pi][:], wp32[2 * DH * pi:
                                                       2 * DH * (pi + 1), :])

            ident = res.tile([P, P], F32, name="ident", tag="ident")
            nc.gpsimd.memset(ident[:], 0.0)
            nc.gpsimd.affine_select(
                out=ident[:], in_=ident[:],
                compare_op=mybir.AluOpType.not_equal, fill=1.0,
                base=0, pattern=[[-1, P]], channel_multiplier=1,
            )

            # causal additive mask for the diagonal 128x128 block:
            # cmask[k, c] = 0 if c >= k else NEG
            cmask = res.tile([P, P], F32, name="cmask", tag="cmask")
            nc.gpsimd.memset(cmask[:], 0.0)
            nc.gpsimd.affine_select(
                out=cmask[:], in_=cmask[:],
                compare_op=mybir.AluOpType.is_ge, fill=NEG,
                base=0, pattern=[[1, P]], channel_multiplier=-1,
            )

            # ---- per-batch resident activations ----
            xT = [res.tile([P, t], FA, name=f"xT{b}", tag=f"xT{b}")
                  for b in range(bpc)]
            qT = [res.tile([P, t], FA, name=f"qT{b}", tag=f"qT{b}")
                  for b in range(bpc)]
            kT = [res.tile([P, t], FA, name=f"kT{b}", tag=f"kT{b}")
                  for b in range(bpc)]
            # V' per (b, kc, h): [128 kpos, 128] = [V_h | ones32 | zeros];
            # ones block at cols 32:64 -> attn@V emits softmax sums on
            # psum rows 32:64 (same tile as y rows 0:32)
            vP = [res.tile([P, nkc, H, P], FA, name=f"vp{b}", tag=f"vp{b}")
                  for b in range(bpc)]
            for b in range(bpc):
                nc.gpsimd.memset(vP[b][:], 0.0)
                nc.gpsimd.memset(vP[b][:, :, :, DH:2 * DH], 1.0)

            # fp32 shadows for the exact recompute of query rows 0:128
            if recompute:
                xT32 = [res.tile([P, P], F32, name=f"xT32_{b}",
                                 tag=f"xT32_{b}") for b in range(bpc)]
                qT32 = [res.tile([P, P], F32, name=f"qT32_{b}",
                                 tag=f"qT32_{b}") for b in range(bpc)]
                kT32 = [res.tile([P, P], F32, name=f"kT32_{b}",
                                 tag=f"kT32_{b}") for b in range(bpc)]
                vP32 = [res.tile([P, H, P], F32, name=f"vP32_{b}",
                                 tag=f"vP32_{b}") for b in range(bpc)]
                for b in range(bpc):
                    nc.gpsimd.memset(vP32[b][:], 0.0)
                    nc.gpsimd.memset(vP32[b][:, :, DH:2 * DH], 1.0)

            import contextlib
            loop_cm = (tc.For_i(0, loop_n, 1) if loop_n
                       else contextlib.nullcontext())
            with loop_cm:

                # ============ phase A (emitted in pieces) ============
                def emit_A_transpose(b, j):
                    """transpose chunks 4j..4j+3 of x into xT[b]."""
                    for kc in range(4 * j, 4 * j + 4):
                        xi = xin_pool.tile([P, D], F32, name="xin",
                                           tag="xin")
                        nc.sync.dma_start(xi[:],
                                          x_d[b, kc * P:(kc + 1) * P, :])
                        pst = ps_pool.tile([P, 2, qg], F32, name="pst",
                                           tag="ps")
                        nc.tensor.transpose(pst[:, 0, 0:P], xi[:], ident[:])
                        nc.vector.tensor_copy(
                            xT[b][:, kc * P:(kc + 1) * P], pst[:, 0, 0:P])
                        if recompute and kc == 0:
                            nc.vector.tensor_copy(xT32[b][:],
                                                  pst[:, 0, 0:P])

                def emit_A_qkv(b, j):
                    """QKV^T for token block j (512 wide)."""
                    sl = slice(j * qg, (j + 1) * qg)
                    for wofs, dst in ((0, qT[b]), (D, kT[b])):
                        pq = ps_pool.tile([P, 2, qg], F32, name="pq",
                                          tag="ps")
                        nc.tensor.matmul(
                            pq[:, 0, :], wa16[:, wofs:wofs + D],
                            xT[b][:, sl], start=True, stop=True)
                        nc.vector.tensor_copy(dst[:, sl], pq[:, 0, :])
                    for kc in range(4 * j, 4 * j + 4):
                        pv = ps_pool.tile([P, 2, qg], F32, name="pv",
                                          tag="ps")
                        nc.tensor.matmul(
                            pv[:, 0, 0:P], xT[b][:, kc * P:(kc + 1) * P],
                            wa16[:, 2 * D:3 * D], start=True, stop=True)
                        nc.vector.tensor_copy(
                            vP[b][:, kc, :, 0:DH],
                            pv[:, 0, 0:P].rearrange("p (h d) -> p h d",
                                                    h=H))
                        if recompute and kc == 0:
                            nc.vector.tensor_copy(
                                vP32[b][:, :, 0:DH],
                                pv[:, 0, 0:P].rearrange("p (h d) -> p h d",
                                                        h=H))

                def emit_A_recompute_qk(b):
                    """exact fp32 q/k for the first 128 tokens."""
                    for wofs, dst in ((0, qT32[b]), (D, kT32[b])):
                        pq = ps_pool.tile([P, 2, qg], F32, name="pq32",
                                          tag="ps")
                        nc.tensor.matmul(
                            pq[:, 0, 0:P], wa32[:, wofs:wofs + D],
                            xT32[b][:], start=True, stop=True)
                        nc.vector.tensor_copy(dst[:], pq[:, 0, 0:P])

                # ====== phase B: software-pipelined chunk stream ======
                # Engines execute their queues in program order, so the
                # emission order IS the per-engine schedule. Producers
                # (scores->mask->exp) for chunk i are emitted together;
                # the consumer (attn@V) for chunk i-DEPTH follows, so the
                # PE never sits in-order-blocked on ACT's exp.
                DEPTH = 2
                y2s = {}    # (b,j,pi) -> y2 psum tile (alloc at 1st cons.)
                ysbs = {}   # (b,j) -> [ysb_pi0, ysb_pi1]

                def emit_P(b, j, pi, kc):
                    ksl = slice(kc * P, (kc + 1) * P)
                    r = kc - cpq * j  # diag band index (>=0: on band)
                    c0 = max(r, 0) * P
                    quad = ps_pool.tile([P, 2, qg], F32, name="quad",
                                        tag="ps")
                    for ci in range(2):
                        h = 2 * pi + ci
                        hp = slice(32 * h, 32 * h + 32)
                        nc.tensor.matmul(
                            quad[:, ci, c0:], kT[b][hp, ksl],
                            qT[b][hp, j * qg + c0:(j + 1) * qg],
                            start=True, stop=True,
                            tile_position=(32 * h, 0))
                    et = exp_pool.tile([P, 2, qg], FA, name="et", tag="et")
                    if r >= 0:
                        blk = slice(c0, c0 + P)
                        nc.vector.tensor_tensor(
                            quad[:, :, blk], quad[:, :, blk],
                            cmask[:, None, :].to_broadcast((P, 2, P)),
                            mybir.AluOpType.add)
                    nc.scalar.activation(
                        et[:, :, c0:], quad[:, :, c0:],
                        mybir.ActivationFunctionType.Exp, scale=scale)
                    return et

                def emit_C(b, j, pi, kc, et):
                    kmax = cpq * (j + 1) - 1
                    c0 = max(kc - cpq * j, 0) * P
                    if kc == 0:
                        y2s[(b, j, pi)] = y_pool.tile([P, 2, qg], F32,
                                                      name="y", tag="y")
                    y2 = y2s[(b, j, pi)]
                    for ci in range(2):
                        h = 2 * pi + ci
                        nc.tensor.matmul(
                            y2[:, ci, c0:], vP[b][:, kc, h, :],
                            et[:, ci, c0:],
                            start=(kc == 0), stop=(kc == kmax),
                            skip_group_check=True)

                def emit_tail(b, j, pi):
                    """recompute (j==0), normalize, proj (after pi==1)."""
                    y2 = y2s.pop((b, j, pi))
                    if recompute and j == 0:
                        q32 = ps_pool.tile([P, 2, qg], F32, name="q32",
                                           tag="ps")
                        for ci in range(2):
                            h = 2 * pi + ci
                            hp = slice(32 * h, 32 * h + 32)
                            nc.tensor.matmul(
                                q32[:, ci, 0:P], kT32[b][hp, :],
                                qT32[b][hp, :], start=True, stop=True,
                                tile_position=(32 * h, 0))
                        nc.vector.tensor_tensor(
                            q32[:, :, 0:P], q32[:, :, 0:P],
                            cmask[:, None, :].to_broadcast((P, 2, P)),
                            mybir.AluOpType.add)
                        et32 = r_pool.tile([P, 2, P], F32, name="et32",
                                           tag="et32")
                        nc.scalar.activation(
                            et32[:], q32[:, :, 0:P],
                            mybir.ActivationFunctionType.Exp, scale=scale)
                        for ci in range(2):
                            h = 2 * pi + ci
                            nc.tensor.matmul(
                                y2[:, ci, 0:P], vP32[b][:, h, :],
                                et32[:, ci, :],
                                start=True, stop=True,
                                skip_group_check=True)
                    # normalize: y rows 0:32, sums rows 32:64
                    rec = rec_pool.tile([DH, 2, qg], F32, name="rec",
                                        tag="rec")
                    nc.vector.reciprocal(rec[:], y2[DH:2 * DH, :, :])
                    ysb = ysb_pool.tile([2 * DH, qg], FA, name="ysb",
                                        tag="ysb")
                    nc.vector.tensor_mul(ysb[0:DH, :], y2[0:DH, 0, :],
                                         rec[:, 0, :])
                    nc.vector.tensor_mul(ysb[DH:2 * DH, :], y2[0:DH, 1, :],
                                         rec[:, 1, :])
                    ysbs.setdefault((b, j), []).append(ysb)
                    if pi == 1:
                        pair = ysbs.pop((b, j))
                        for tch in range(cpq):
                            t0 = j * qg + tch * P
                            csl = slice(tch * P, (tch + 1) * P)
                            po = ps_pool.tile([P, 2, qg], F32, name="proj",
                                              tag="ps")
                            for pp in range(2):
                                nc.tensor.matmul(
                                    po[:, 0, 0:D], pair[pp][:, csl],
                                    wpP[pp][:],
                                    start=(pp == 0), stop=(pp == 1),
                                    skip_group_check=True)
                            ob = out_pool.tile([P, D], F32, name="ob",
                                               tag="ob")
                            nc.vector.tensor_copy(ob[:], po[:, 0, 0:D])
                            nc.sync.dma_start(out_d[b, t0:t0 + P, :],
                                              ob[:])

                def emit_A_head(b):
                    emit_A_transpose(b, 0)
                    emit_A_qkv(b, 0)
                    if recompute:
                        emit_A_recompute_qk(b)

                def emit_B_stream(bs, pieces):
                    """Pipelined stream over all (b, j, pi, kc) of the
                    given batches; `pieces` are phase-A callables woven
                    in at group boundaries."""
                    items = [(b, j, pi, kc)
                             for b in bs
                             for j in range(nqg)
                             for pi in range(2)
                             for kc in range(cpq * (j + 1))]
                    pend = []     # (item, et) awaiting consumption
                    pi_q = list(pieces)

                    def consume_one():
                        (bb, jj, pp, kk), et = pend.pop(0)
                        emit_C(bb, jj, pp, kk, et)
                        if kk == cpq * (jj + 1) - 1:  # group done
                            emit_tail(bb, jj, pp)
                            if pi_q:
                                pi_q.pop(0)()

                    for it in items:
                        et = emit_P(*it)
                        pend.append((it, et))
                        if len(pend) > DEPTH:
                            consume_one()
                    while pend:
                        consume_one()
                    while pi_q:
                        pi_q.pop(0)()

                if interleave and bpc == 2:
                    emit_A_head(0)
                    pieces = [lambda j=j: (emit_A_transpose(0, j),
                                           emit_A_qkv(0, j))
                              for j in range(1, nqg)]
                    pieces += [lambda: emit_A_head(1)]
                    pieces += [lambda j=j: (emit_A_transpose(1, j),
                                            emit_A_qkv(1, j))
                               for j in range(1, nqg)]
                    emit_B_stream([0], pieces)
                    emit_B_stream([1], [])
                else:
                    for b in range(bpc):
                        emit_A_head(b)
                        for j in range(1, nqg):
                            emit_A_transpose(b, j)
                            emit_A_qkv(b, j)
                    for b in range(bpc):
                        emit_B_stream([b], [])
    nc.compile()
    return nc


_NC_CACHE = {}

CONFIG = {"attn_dt": "f16", "recompute": True, "interleave": True,
          "quad_bufs": 2, "exp_bufs": 6}


def _get_nc(bpc=BPC, t=T, loop_n=0):
    key = (bpc, t, loop_n)
    if key not in _NC_CACHE:
        _NC_CACHE[key] = build_attention_nc(bpc=bpc, t=t, loop_n=loop_n,
                                            **CONFIG)
    return _NC_CACHE[key]


def _run(x, w_attn, w_proj, **spmd_kwargs):
    x = np.ascontiguousarray(np.asarray(x), dtype=np.float32)
    w_attn = np.ascontiguousarray(np.asarray(w_attn), dtype=np.float32)
    w_proj = np.ascontiguousarray(np.asarray(w_proj), dtype=np.float32)
    assert x.shape == (B, T, D), x.shape

    nc = _get_nc()
    in_maps = [
        {"x": x[c * BPC:(c + 1) * BPC], "w_attn": w_attn, "w_proj": w_proj}
        for c in range(N_CORES)
    ]
    res = run_bass_kernel_spmd(nc, in_maps, list(range(N_CORES)),
                               **spmd_kwargs)
    out = np.concatenate([res.results[c]["out"] for c in range(N_CORES)],
                         axis=0)
    return out.astype(np.float32), res


def kernel(x, w_attn, w_proj):
    out, _ = _run(x, w_attn, w_proj)
    return out


if __name__ == "__main__":
    nc = build_attention_nc()
    print("built ok")


# revision 10
# speedup vs baseline: 1.2094x; 1.0091x over previous
"""Causal self-attention Trainium2 Bass kernel (v2).

Problem: B=16, T=2048, D=128, H=4 (head dim 32), fp32 in/out.
  qkv = x @ w_attn ; per-head scores = q k^T / sqrt(32), causal
  y = softmax(scores) @ v ; out = y @ w_proj

Sharding: data-parallel over batch, 2 batches per NeuronCore x 8 cores
(SPMD, no collectives).

v2 design (vs the 428us baseline):
  - ScalarE (exp) is the roofline engine (~150us serial of exp work per
    core); everything else is structured to overlap under it.
  - denominator trick: V' per head = [V_h | ones32 | zeros] so attn@V
    puts softmax sums on PSUM rows 32:64 of the same tile as y (rows
    0:32); normalization is one partition-shifted DVE reciprocal + two
    muls. No expander matmul, no sums staging copies.
  - fp16 operands everywhere on the PE (QKV projection, scores, attn@V,
    out projection); fp32 only for the exact recompute of query rows
    0:128 (they attend over few keys, fp16 noise is not averaged away).
  - diagonal-band trimming: scores matmul, exp, and attn@V all stream
    only columns >= r*128 of a diag chunk; nothing reads the masked
    region so no et zero-fill is needed.
  - out projection contracts K=64 (pair's 64 live ysb rows): no zero
    padding to maintain, pooled ysb tiles.
  - single pool scope; emission interleaves batch-1 phase A into
    batch-0 phase B so QKV/transpose PE+DVE work fills gaps under ACT.

Hardware traps (TRN2, carried over from baseline):
  - M<128 col-tiled matmuls (tile_position=(0,64)) crash / corrupt;
    all attn@V matmuls use M=128, out-proj uses K=64 at (0,0).
  - accumulating K=32 matmuls at different row positions into one PSUM
    bank crash; scores quads use start/stop=True per chunk.
  - engine partition offsets must be 32-aligned; 32-step cross-operand
    partition shifts are legal on DVE (used by reciprocal/mul).
"""

import os
import sys

import numpy as np

for _p in ("/opt/trn_rl_repo", "/root/.axon_site/_ro/trn_rl_repo"):
    if os.path.isdir(_p) and _p not in sys.path:
        sys.path.insert(0, _p)

import concourse.bass as bass
import concourse.bacc as bacc
import concourse.mybir as mybir
import concourse.tile as tile
from concourse.bass_utils import run_bass_kernel_spmd

F32 = mybir.dt.float32
F16 = mybir.dt.float16
P = 128
NEG = -1.0e9

B, T, D, H = 16, 2048, 128, 4
DH = D // H  # 32
N_CORES = 8
BPC = B // N_CORES  # batches per core


def build_attention_nc(bpc=BPC, t=T, qg=512, loop_n=0, attn_dt="f16",
                       recompute=True, interleave=True, quad_bufs=2,
                       exp_bufs=6, ablate=()):
    ablate = set(ablate)
    assert t % qg == 0 and qg % P == 0 and t % P == 0
    nqg = t // qg      # q groups
    nkc = t // P       # 128-wide k chunks
    cpq = qg // P      # k chunks per q group
    scale = 1.0 / float(np.sqrt(DH))
    FA = {"f16": F16, "f32": F32}[attn_dt]

    nc = bacc.Bacc("TRN2", target_bir_lowering=False, debug=False)
    x_d = nc.dram_tensor("x", [bpc, t, D], F32, kind="ExternalInput")
    wa_d = nc.dram_tensor("w_attn", [D, 3 * D], F32, kind="ExternalInput")
    wp_d = nc.dram_tensor("w_proj", [D, D], F32, kind="ExternalInput")
    out_d = nc.dram_tensor("out", [bpc, t, D], F32, kind="ExternalOutput")

    with tile.TileContext(nc) as tc:
        with (
            tc.tile_pool(name="resident", bufs=1) as res,
            tc.tile_pool(name="xin", bufs=8) as xin_pool,
            tc.tile_pool(name="ps", bufs=quad_bufs, space="PSUM") as ps_pool,
            tc.tile_pool(name="ypsum", bufs=2, space="PSUM") as y_pool,
            tc.tile_pool(name="expt", bufs=exp_bufs) as exp_pool,
            tc.tile_pool(name="recp", bufs=3) as rec_pool,
            tc.tile_pool(name="ysb", bufs=4) as ysb_pool,
            tc.tile_pool(name="outsb", bufs=6) as out_pool,
            tc.tile_pool(name="r32", bufs=2) as r_pool,
        ):
            # ---- resident constants ----
            wa32 = res.tile([D, 3 * D], F32, name="wa32", tag="wa32")
            nc.sync.dma_start(wa32[:], wa_d[:])
            wa16 = res.tile([D, 3 * D], FA, name="wa16", tag="wa16")
            nc.vector.tensor_copy(wa16[:], wa32[:])

            # w_proj: pair pi's tile holds head 2pi rows at 0:32 and head
            # 2pi+1 rows at 32:64 (out-proj contracts K=64)
            wp32 = res.tile([D, D], F32, name="wp32", tag="wp32")
            nc.sync.dma_start(wp32[:], wp_d[:])
            wpP = [res.tile([2 * DH, D], FA, name=f"wp{pi}", tag=f"wp{pi}")
                   for pi in range(2)]
            for pi in range(2):
                nc.vector.tensor_copy(wpP[pi][:], wp32[2 * DH * pi:
                                                       2 * DH * (pi + 1), :])

            ident = res.tile([P, P], F32, name="ident", tag="ident")
            nc.gpsimd.memset(ident[:], 0.0)
            nc.gpsimd.affine_select(
                out=ident[:], in_=ident[:],
                compare_op=mybir.AluOpType.not_equal, fill=1.0,
                base=0, pattern=[[-1, P]], channel_multiplier=1,
            )

            # causal additive mask for the diagonal 128x128 block:
            # cmask[k, c] = 0 if c >= k else NEG
            cmask = res.tile([P, P], F32, name="cmask", tag="cmask")
            nc.gpsimd.memset(cmask[:], 0.0)
            nc.gpsimd.affine_select(
                out=cmask[:], in_=cmask[:],
                compare_op=mybir.AluOpType.is_ge, fill=NEG,
                base=0, pattern=[[1, P]], channel_multiplier=-1,
            )

            # ---- per-batch resident activations ----
            xT = [res.tile([P, t], FA, name=f"xT{b}", tag=f"xT{b}")
                  for b in range(bpc)]
            qT = [res.tile([P, t], FA, name=f"qT{b}", tag=f"qT{b}")
                  for b in range(bpc)]
            kT = [res.tile([P, t], FA, name=f"kT{b}", tag=f"kT{b}")
                  for b in range(bpc)]
            # V' per (b, kc, h): [128 kpos, 128] = [V_h | ones32 | zeros];
            # ones block at cols 32:64 -> attn@V emits softmax sums on
            # psum rows 32:64 (same tile as y rows 0:32)
            vP = [res.tile([P, nkc, H, P], FA, name=f"vp{b}", tag=f"vp{b}")
                  for b in range(bpc)]
            for b in range(bpc):
                nc.gpsimd.memset(vP[b][:], 0.0)
                nc.gpsimd.memset(vP[b][:, :, :, DH:2 * DH], 1.0)

            # fp32 shadows for the exact recompute of query rows 0:128
            if recompute:
                xT32 = [res.tile([P, P], F32, name=f"xT32_{b}",
                                 tag=f"xT32_{b}") for b in range(bpc)]
                qT32 = [res.tile([P, P], F32, name=f"qT32_{b}",
                                 tag=f"qT32_{b}") for b in range(bpc)]
                kT32 = [res.tile([P, P], F32, name=f"kT32_{b}",
                                 tag=f"kT32_{b}") for b in range(bpc)]
                vP32 = [res.tile([P, H, P], F32, name=f"vP32_{b}",
                                 tag=f"vP32_{b}") for b in range(bpc)]
                for b in range(bpc):
                    nc.gpsimd.memset(vP32[b][:], 0.0)
                    nc.gpsimd.memset(vP32[b][:, :, DH:2 * DH], 1.0)

            import contextlib
            loop_cm = (tc.For_i(0, loop_n, 1) if loop_n
                       else contextlib.nullcontext())
            with loop_cm:

                # ============ phase A (emitted in pieces) ============
                def emit_A_transpose(b, j):
                    """transpose chunks 4j..4j+3 of x into xT[b]."""
                    for kc in range(4 * j, 4 * j + 4):
                        xi = xin_pool.tile([P, D], F32, name="xin",
                                           tag="xin")
                        nc.sync.dma_start(xi[:],
                                          x_d[b, kc * P:(kc + 1) * P, :])
                        pst = ps_pool.tile([P, 2, qg], F32, name="pst",
                                           tag="ps")
                        nc.tensor.transpose(pst[:, 0, 0:P], xi[:], ident[:])
                        nc.vector.tensor_copy(
                            xT[b][:, kc * P:(kc + 1) * P], pst[:, 0, 0:P])
                        if recompute and kc == 0:
                            nc.vector.tensor_copy(xT32[b][:],
                                                  pst[:, 0, 0:P])

                def emit_A_qkv(b, j):
                    """QKV^T for token block j (512 wide)."""
                    sl = slice(j * qg, (j + 1) * qg)
                    for wofs, dst in ((0, qT[b]), (D, kT[b])):
                        pq = ps_pool.tile([P, 2, qg], F32, name="pq",
                                          tag="ps")
                        nc.tensor.matmul(
                            pq[:, 0, :], wa16[:, wofs:wofs + D],
                            xT[b][:, sl], start=True, stop=True)
                        nc.vector.tensor_copy(dst[:, sl], pq[:, 0, :])
                    for kc in range(4 * j, 4 * j + 4):
                        pv = ps_pool.tile([P, 2, qg], F32, name="pv",
                                          tag="ps")
                        nc.tensor.matmul(
                            pv[:, 0, 0:P], xT[b][:, kc * P:(kc + 1) * P],
                            wa16[:, 2 * D:3 * D], start=True, stop=True)
                        nc.vector.tensor_copy(
                            vP[b][:, kc, :, 0:DH],
                            pv[:, 0, 0:P].rearrange("p (h d) -> p h d",
                                                    h=H))
                        if recompute and kc == 0:
                            nc.vector.tensor_copy(
                                vP32[b][:, :, 0:DH],
                                pv[:, 0, 0:P].rearrange("p (h d) -> p h d",
                                                        h=H))

                def emit_A_recompute_qk(b):
                    """exact fp32 q/k for the first 128 tokens."""
                    for wofs, dst in ((0, qT32[b]), (D, kT32[b])):
                        pq = ps_pool.tile([P, 2, qg], F32, name="pq32",
                                          tag="ps")
                        nc.tensor.matmul(
                            pq[:, 0, 0:P], wa32[:, wofs:wofs + D],
                            xT32[b][:], start=True, stop=True)
                        nc.vector.tensor_copy(dst[:], pq[:, 0, 0:P])

                # ====== phase B: software-pipelined chunk stream ======
                # Engines execute their queues in program order, so the
                # emission order IS the per-engine schedule. Producers
                # (scores->mask->exp) for chunk i are emitted together;
                # the consumer (attn@V) for chunk i-DEPTH follows, so the
                # PE never sits in-order-blocked on ACT's exp.
                DEPTH = 2
                y2s = {}    # (b,j,pi) -> y2 psum tile (alloc at 1st cons.)
                ysbs = {}   # (b,j) -> [ysb_pi0, ysb_pi1]

                def emit_P(b, j, pi, kc):
                    ksl = slice(kc * P, (kc + 1) * P)
                    r = kc - cpq * j  # diag band index (>=0: on band)
                    c0 = max(r, 0) * P
                    quad = ps_pool.tile([P, 2, qg], F32, name="quad",
                                        tag="ps")
                    s_end = (c0 + P if "scores128" in ablate
                             else qg)
                    if "no_scores" not in ablate:
                        for ci in range(2):
                            h = 2 * pi + ci
                            hp = slice(32 * h, 32 * h + 32)
                            nc.tensor.matmul(
                                quad[:, ci, c0:s_end], kT[b][hp, ksl],
                                qT[b][hp, j * qg + c0:j * qg + s_end],
                                start=True, stop=True,
                                tile_position=(32 * h, 0))
                    et = exp_pool.tile([P, 2, qg], FA, name="et", tag="et")
                    if r >= 0 and "no_mask" not in ablate:
                        blk = slice(c0, c0 + P)
                        nc.vector.tensor_tensor(
                            quad[:, :, blk], quad[:, :, blk],
                            cmask[:, None, :].to_broadcast((P, 2, P)),
                            mybir.AluOpType.add)
                    e_end = c0 + P if "exp128" in ablate else qg
                    nc.scalar.activation(
                        et[:, :, c0:e_end], quad[:, :, c0:e_end],
                        mybir.ActivationFunctionType.Exp, scale=scale)
                    return et

                def emit_C(b, j, pi, kc, et):
                    kmax = cpq * (j + 1) - 1
                    c0 = max(kc - cpq * j, 0) * P
                    if kc == 0:
                        y2s[(b, j, pi)] = y_pool.tile([P, 2, qg], F32,
                                                      name="y", tag="y")
                    y2 = y2s[(b, j, pi)]
                    a_end = c0 + P if "attnv128" in ablate else qg
                    for ci in range(2):
                        h = 2 * pi + ci
                        nc.tensor.matmul(
                            y2[:, ci, c0:a_end], vP[b][:, kc, h, :],
                            et[:, ci, c0:a_end],
                            start=(kc == 0), stop=(kc == kmax),
                            skip_group_check=True)

                def emit_tail(b, j, pi):
                    """recompute (j==0), normalize, proj (after pi==1)."""
                    y2 = y2s.pop((b, j, pi))
                    if "no_tail" in ablate:
                        return
                    if recompute and j == 0:
                        q32 = ps_pool.tile([P, 2, qg], F32, name="q32",
                                           tag="ps")
                        for ci in range(2):
                            h = 2 * pi + ci
                            hp = slice(32 * h, 32 * h + 32)
                            nc.tensor.matmul(
                                q32[:, ci, 0:P], kT32[b][hp, :],
                                qT32[b][hp, :], start=True, stop=True,
                                tile_position=(32 * h, 0))
                        nc.vector.tensor_tensor(
                            q32[:, :, 0:P], q32[:, :, 0:P],
                            cmask[:, None, :].to_broadcast((P, 2, P)),
                            mybir.AluOpType.add)
                        et32 = r_pool.tile([P, 2, P], F32, name="et32",
                                           tag="et32")
                        nc.scalar.activation(
                            et32[:], q32[:, :, 0:P],
                            mybir.ActivationFunctionType.Exp, scale=scale)
                        for ci in range(2):
                            h = 2 * pi + ci
                            nc.tensor.matmul(
                                y2[:, ci, 0:P], vP32[b][:, h, :],
                                et32[:, ci, :],
                                start=True, stop=True,
                                skip_group_check=True)
                    # normalize: y rows 0:32, sums rows 32:64
                    rec = rec_pool.tile([DH, 2, qg], F32, name="rec",
                                        tag="rec")
                    nc.vector.reciprocal(rec[:], y2[DH:2 * DH, :, :])
                    ysb = ysb_pool.tile([2 * DH, qg], FA, name="ysb",
                                        tag="ysb")
                    nc.vector.tensor_mul(ysb[0:DH, :], y2[0:DH, 0, :],
                                         rec[:, 0, :])
                    nc.vector.tensor_mul(ysb[DH:2 * DH, :], y2[0:DH, 1, :],
                                         rec[:, 1, :])
                    ysbs.setdefault((b, j), []).append(ysb)
                    if pi == 1:
                        pair = ysbs.pop((b, j))
                        for tch in range(cpq):
                            t0 = j * qg + tch * P
                            csl = slice(tch * P, (tch + 1) * P)
                            po = ps_pool.tile([P, 2, qg], F32, name="proj",
                                              tag="ps")
                            for pp in range(2):
                                nc.tensor.matmul(
                                    po[:, 0, 0:D], pair[pp][:, csl],
                                    wpP[pp][:],
                                    start=(pp == 0), stop=(pp == 1),
                                    skip_group_check=True)
                            ob = out_pool.tile([P, D], F32, name="ob",
                                               tag="ob")
                            nc.vector.tensor_copy(ob[:], po[:, 0, 0:D])
                            nc.sync.dma_start(out_d[b, t0:t0 + P, :],
                                              ob[:])

                def emit_A_head(b):
                    emit_A_transpose(b, 0)
                    emit_A_qkv(b, 0)
                    if recompute:
                        emit_A_recompute_qk(b)

                def emit_B_stream(bs, pieces, depth=DEPTH):
                    """Pipelined stream over all (b, j, pi, kc) of the
                    given batches (lockstep-interleaved when several);
                    `pieces` are phase-A callables woven in at group
                    boundaries."""
                    per_b = [[(b, j, pi, kc)
                              for j in range(nqg)
                              for pi in range(2)
                              for kc in range(cpq * (j + 1))]
                             for b in bs]
                    items = [it for tup in zip(*per_b) for it in tup]
                    pend = []     # (item, et) awaiting consumption
                    pi_q = list(pieces)

                    def consume_one():
                        (bb, jj, pp, kk), et = pend.pop(0)
                        emit_C(bb, jj, pp, kk, et)
                        if kk == cpq * (jj + 1) - 1:  # group done
                            emit_tail(bb, jj, pp)
                            if pi_q:
                                pi_q.pop(0)()

                    for it in items:
                        et = emit_P(*it)
                        pend.append((it, et))
                        if len(pend) > depth:
                            consume_one()
                    while pend:
                        consume_one()
                    while pi_q:
                        pi_q.pop(0)()

                if interleave == "batch" and bpc == 2:
                    # all phase A up front (dense PE work warms the HAM
                    # clock gate), then both batches' attention streams
                    # in lockstep: PE always has the other batch's
                    # matmuls while ACT runs this batch's exp.
                    for b in range(bpc):
                        emit_A_head(b)
                        for j in range(1, nqg):
                            emit_A_transpose(b, j)
                            emit_A_qkv(b, j)
                    emit_B_stream([0, 1], [], depth=2 * DEPTH)
                elif interleave and bpc == 2:
                    emit_A_head(0)
                    pieces = [lambda j=j: (emit_A_transpose(0, j),
                                           emit_A_qkv(0, j))
                              for j in range(1, nqg)]
                    pieces += [lambda: emit_A_head(1)]
                    pieces += [lambda j=j: (emit_A_transpose(1, j),
                                            emit_A_qkv(1, j))
                               for j in range(1, nqg)]
                    emit_B_stream([0], pieces)
                    emit_B_stream([1], [])
                else:
                    for b in range(bpc):
                        emit_A_head(b)
                        for j in range(1, nqg):
                            emit_A_transpose(b, j)
                            emit_A_qkv(b, j)
                    for b in range(bpc):
                        emit_B_stream([b], [])
    nc.compile()
    return nc


_NC_CACHE = {}

CONFIG = {"attn_dt": "f16", "recompute": True, "interleave": "batch",
          "quad_bufs": 2, "exp_bufs": 8}


def _get_nc(bpc=BPC, t=T, loop_n=0):
    key = (bpc, t, loop_n)
    if key not in _NC_CACHE:
        _NC_CACHE[key] = build_attention_nc(bpc=bpc, t=t, loop_n=loop_n,
                                            **CONFIG)
    return _NC_CACHE[key]


def _run(x, w_attn, w_proj, **spmd_kwargs):
    x = np.ascontiguousarray(np.asarray(x), dtype=np.float32)
    w_attn = np.ascontiguousarray(np.asarray(w_attn), dtype=np.float32)
    w_proj = np.ascontiguousarray(np.asarray(w_proj), dtype=np.float32)
    assert x.shape == (B, T, D), x.shape

    nc = _get_nc()
    in_maps = [
        {"x": x[c * BPC:(c + 1) * BPC], "w_attn": w_attn, "w_proj": w_proj}
        for c in range(N_CORES)
    ]
    res = run_bass_kernel_spmd(nc, in_maps, list(range(N_CORES)),
                               **spmd_kwargs)
    out = np.concatenate([res.results[c]["out"] for c in range(N_CORES)],
                         axis=0)
    return out.astype(np.float32), res


def kernel(x, w_attn, w_proj):
    out, _ = _run(x, w_attn, w_proj)
    return out


if __name__ == "__main__":
    nc = build_attention_nc()
    print("built ok")


# revision 16
# speedup vs baseline: 1.3143x; 1.0867x over previous
"""Causal self-attention Trainium2 Bass kernel (v2).

Problem: B=16, T=2048, D=128, H=4 (head dim 32), fp32 in/out.
  qkv = x @ w_attn ; per-head scores = q k^T / sqrt(32), causal
  y = softmax(scores) @ v ; out = y @ w_proj

Sharding: data-parallel over batch, 2 batches per NeuronCore x 8 cores
(SPMD, no collectives).

v2 design (vs the 428us baseline):
  - ScalarE (exp) is the roofline engine (~150us serial of exp work per
    core); everything else is structured to overlap under it.
  - denominator trick: V' per head = [V_h | ones32 | zeros] so attn@V
    puts softmax sums on PSUM rows 32:64 of the same tile as y (rows
    0:32); normalization is one partition-shifted DVE reciprocal + two
    muls. No expander matmul, no sums staging copies.
  - fp16 operands everywhere on the PE (QKV projection, scores, attn@V,
    out projection); fp32 only for the exact recompute of query rows
    0:128 (they attend over few keys, fp16 noise is not averaged away).
  - diagonal-band trimming: scores matmul, exp, and attn@V all stream
    only columns >= r*128 of a diag chunk; nothing reads the masked
    region so no et zero-fill is needed.
  - out projection contracts K=64 (pair's 64 live ysb rows): no zero
    padding to maintain, pooled ysb tiles.
  - single pool scope; emission interleaves batch-1 phase A into
    batch-0 phase B so QKV/transpose PE+DVE work fills gaps under ACT.

Hardware traps (TRN2, carried over from baseline):
  - M<128 col-tiled matmuls (tile_position=(0,64)) crash / corrupt;
    all attn@V matmuls use M=128, out-proj uses K=64 at (0,0).
  - accumulating K=32 matmuls at different row positions into one PSUM
    bank crash; scores quads use start/stop=True per chunk.
  - engine partition offsets must be 32-aligned; 32-step cross-operand
    partition shifts are legal on DVE (used by reciprocal/mul).
"""

import os
import sys

import numpy as np

for _p in ("/opt/trn_rl_repo", "/root/.axon_site/_ro/trn_rl_repo"):
    if os.path.isdir(_p) and _p not in sys.path:
        sys.path.insert(0, _p)

import concourse.bass as bass
import concourse.bacc as bacc
import concourse.mybir as mybir
import concourse.tile as tile
from concourse.bass_utils import run_bass_kernel_spmd

F32 = mybir.dt.float32
F16 = mybir.dt.float16
P = 128
NEG = -1.0e9

B, T, D, H = 16, 2048, 128, 4
DH = D // H  # 32
N_CORES = 8
BPC = B // N_CORES  # batches per core


def build_attention_nc(bpc=BPC, t=T, qg=512, loop_n=0, attn_dt="f16",
                       recompute=True, interleave=True, quad_bufs=2,
                       exp_bufs=6, ablate=()):
    ablate = set(ablate)
    assert t % qg == 0 and qg % P == 0 and t % P == 0
    nqg = t // qg      # q groups
    nkc = t // P       # 128-wide k chunks
    cpq = qg // P      # k chunks per q group
    scale = 1.0 / float(np.sqrt(DH))
    FA = {"f16": F16, "f32": F32}[attn_dt]

    nc = bacc.Bacc("TRN2", target_bir_lowering=False, debug=False)
    x_d = nc.dram_tensor("x", [bpc, t, D], F32, kind="ExternalInput")
    wa_d = nc.dram_tensor("w_attn", [D, 3 * D], F32, kind="ExternalInput")
    wp_d = nc.dram_tensor("w_proj", [D, D], F32, kind="ExternalInput")
    out_d = nc.dram_tensor("out", [bpc, t, D], F32, kind="ExternalOutput")

    with tile.TileContext(nc) as tc:
        with (
            tc.tile_pool(name="resident", bufs=1) as res,
            tc.tile_pool(name="xin", bufs=8) as xin_pool,
            tc.tile_pool(name="ps", bufs=quad_bufs, space="PSUM") as ps_pool,
            tc.tile_pool(name="ypsum", bufs=2, space="PSUM") as y_pool,
            tc.tile_pool(name="expt", bufs=exp_bufs) as exp_pool,
            tc.tile_pool(name="recp", bufs=3) as rec_pool,
            tc.tile_pool(name="ysb", bufs=4) as ysb_pool,
            tc.tile_pool(name="outsb", bufs=6) as out_pool,
            tc.tile_pool(name="r32", bufs=2) as r_pool,
        ):
            # ---- resident constants ----
            wa32 = res.tile([D, 3 * D], F32, name="wa32", tag="wa32")
            nc.sync.dma_start(wa32[:], wa_d[:])
            wa16 = res.tile([D, 3 * D], FA, name="wa16", tag="wa16")
            nc.vector.tensor_copy(wa16[:], wa32[:])

            # w_proj: pair pi's tile holds head 2pi rows at 0:32 and head
            # 2pi+1 rows at 32:64 (out-proj contracts K=64)
            wp32 = res.tile([D, D], F32, name="wp32", tag="wp32")
            nc.sync.dma_start(wp32[:], wp_d[:])
            wpP = [res.tile([2 * DH, D], FA, name=f"wp{pi}", tag=f"wp{pi}")
                   for pi in range(2)]
            for pi in range(2):
                nc.vector.tensor_copy(wpP[pi][:], wp32[2 * DH * pi:
                                                       2 * DH * (pi + 1), :])

            ident = res.tile([P, P], F32, name="ident", tag="ident")
            nc.gpsimd.memset(ident[:], 0.0)
            nc.gpsimd.affine_select(
                out=ident[:], in_=ident[:],
                compare_op=mybir.AluOpType.not_equal, fill=1.0,
                base=0, pattern=[[-1, P]], channel_multiplier=1,
            )

            # causal additive mask for the diagonal 128x128 block:
            # cmask[k, c] = 0 if c >= k else NEG
            cmask = res.tile([P, P], F32, name="cmask", tag="cmask")
            nc.gpsimd.memset(cmask[:], 0.0)
            nc.gpsimd.affine_select(
                out=cmask[:], in_=cmask[:],
                compare_op=mybir.AluOpType.is_ge, fill=NEG,
                base=0, pattern=[[1, P]], channel_multiplier=-1,
            )

            # ---- per-batch resident activations ----
            xT = [res.tile([P, t], FA, name=f"xT{b}", tag=f"xT{b}")
                  for b in range(bpc)]
            qT = [res.tile([P, t], FA, name=f"qT{b}", tag=f"qT{b}")
                  for b in range(bpc)]
            kT = [res.tile([P, t], FA, name=f"kT{b}", tag=f"kT{b}")
                  for b in range(bpc)]
            # V' per (b, kc, h): [128 kpos, 128] = [V_h | ones32 | zeros];
            # ones block at cols 32:64 -> attn@V emits softmax sums on
            # psum rows 32:64 (same tile as y rows 0:32)
            vP = [res.tile([P, nkc, H, P], FA, name=f"vp{b}", tag=f"vp{b}")
                  for b in range(bpc)]
            for b in range(bpc):
                nc.gpsimd.memset(vP[b][:], 0.0)
                nc.gpsimd.memset(vP[b][:, :, :, DH:2 * DH], 1.0)

            # fp32 shadows for the exact recompute of query rows 0:128
            if recompute:
                xT32 = [res.tile([P, P], F32, name=f"xT32_{b}",
                                 tag=f"xT32_{b}") for b in range(bpc)]
                qT32 = [res.tile([P, P], F32, name=f"qT32_{b}",
                                 tag=f"qT32_{b}") for b in range(bpc)]
                kT32 = [res.tile([P, P], F32, name=f"kT32_{b}",
                                 tag=f"kT32_{b}") for b in range(bpc)]
                vP32 = [res.tile([P, H, P], F32, name=f"vP32_{b}",
                                 tag=f"vP32_{b}") for b in range(bpc)]
                for b in range(bpc):
                    nc.gpsimd.memset(vP32[b][:], 0.0)
                    nc.gpsimd.memset(vP32[b][:, :, DH:2 * DH], 1.0)

            import contextlib
            loop_cm = (tc.For_i(0, loop_n, 1) if loop_n
                       else contextlib.nullcontext())
            with loop_cm:

                # ============ phase A (emitted in pieces) ============
                def emit_A_transpose(b, j):
                    """transpose chunks 4j..4j+3 of x into xT[b]."""
                    for kc in range(4 * j, 4 * j + 4):
                        xi = xin_pool.tile([P, D], F32, name="xin",
                                           tag="xin")
                        nc.sync.dma_start(xi[:],
                                          x_d[b, kc * P:(kc + 1) * P, :])
                        pst = ps_pool.tile([P, 2, qg], F32, name="pst",
                                           tag="ps")
                        nc.tensor.transpose(pst[:, 0, 0:P], xi[:], ident[:])
                        # ACT is idle in phase A; offload these copies
                        nc.scalar.copy(
                            xT[b][:, kc * P:(kc + 1) * P], pst[:, 0, 0:P])
                        if recompute and kc == 0:
                            nc.vector.tensor_copy(xT32[b][:],
                                                  pst[:, 0, 0:P])

                def emit_A_qkv(b, j):
                    """QKV^T for token block j (512 wide)."""
                    sl = slice(j * qg, (j + 1) * qg)
                    for wofs, dst in ((0, qT[b]), (D, kT[b])):
                        pq = ps_pool.tile([P, 2, qg], F32, name="pq",
                                          tag="ps")
                        nc.tensor.matmul(
                            pq[:, 0, :], wa16[:, wofs:wofs + D],
                            xT[b][:, sl], start=True, stop=True)
                        nc.vector.tensor_copy(dst[:, sl], pq[:, 0, :])
                    for kc in range(4 * j, 4 * j + 4):
                        pv = ps_pool.tile([P, 2, qg], F32, name="pv",
                                          tag="ps")
                        nc.tensor.matmul(
                            pv[:, 0, 0:P], xT[b][:, kc * P:(kc + 1) * P],
                            wa16[:, 2 * D:3 * D], start=True, stop=True)
                        cp = (nc.vector.tensor_copy if kc % 2 == 0
                              else nc.scalar.copy)
                        cp(vP[b][:, kc, :, 0:DH],
                           pv[:, 0, 0:P].rearrange("p (h d) -> p h d",
                                                   h=H))
                        if recompute and kc == 0:
                            nc.vector.tensor_copy(
                                vP32[b][:, :, 0:DH],
                                pv[:, 0, 0:P].rearrange("p (h d) -> p h d",
                                                        h=H))

                def emit_A_recompute_qk(b):
                    """exact fp32 q/k for the first 128 tokens."""
                    for wofs, dst in ((0, qT32[b]), (D, kT32[b])):
                        pq = ps_pool.tile([P, 2, qg], F32, name="pq32",
                                          tag="ps")
                        nc.tensor.matmul(
                            pq[:, 0, 0:P], wa32[:, wofs:wofs + D],
                            xT32[b][:], start=True, stop=True)
                        nc.vector.tensor_copy(dst[:], pq[:, 0, 0:P])

                # ====== phase B: software-pipelined chunk stream ======
                # Engines execute their queues in program order, so the
                # emission order IS the per-engine schedule. Producers
                # (scores->mask->exp) for chunk i are emitted together;
                # the consumer (attn@V) for chunk i-DEPTH follows, so the
                # PE never sits in-order-blocked on ACT's exp.
                DEPTH = 2
                y2s = {}    # (b,j,pi) -> y2 psum tile (alloc at 1st cons.)
                ysbs = {}   # (b,j) -> [ysb_pi0, ysb_pi1]

                def emit_P(b, j, pi, kc):
                    ksl = slice(kc * P, (kc + 1) * P)
                    r = kc - cpq * j  # diag band index (>=0: on band)
                    c0 = max(r, 0) * P
                    quad = ps_pool.tile([P, 2, qg], F32, name="quad",
                                        tag="ps")
                    s_end = (c0 + P if "scores128" in ablate
                             else qg)
                    if "no_scores" not in ablate:
                        for ci in range(2):
                            h = 2 * pi + ci
                            hp = slice(32 * h, 32 * h + 32)
                            nc.tensor.matmul(
                                quad[:, ci, c0:s_end], kT[b][hp, ksl],
                                qT[b][hp, j * qg + c0:j * qg + s_end],
                                start=True, stop=True,
                                tile_position=(32 * h, 0))
                    et = exp_pool.tile([P, 2, qg], FA, name="et", tag="et")
                    if r >= 0 and "no_mask" not in ablate:
                        blk = slice(c0, c0 + P)
                        nc.vector.tensor_tensor(
                            quad[:, :, blk], quad[:, :, blk],
                            cmask[:, None, :].to_broadcast((P, 2, P)),
                            mybir.AluOpType.add)
                    e_end = c0 + P if "exp128" in ablate else qg
                    nc.scalar.activation(
                        et[:, :, c0:e_end], quad[:, :, c0:e_end],
                        mybir.ActivationFunctionType.Exp, scale=scale)
                    return et

                def emit_C(b, j, pi, kc, et):
                    kmax = cpq * (j + 1) - 1
                    c0 = max(kc - cpq * j, 0) * P
                    if kc == 0:
                        y2s[(b, j, pi)] = y_pool.tile([P, 2, qg], F32,
                                                      name="y", tag="y")
                    y2 = y2s[(b, j, pi)]
                    a_end = c0 + P if "attnv128" in ablate else qg
                    for ci in range(2):
                        h = 2 * pi + ci
                        nc.tensor.matmul(
                            y2[:, ci, c0:a_end], vP[b][:, kc, h, :],
                            et[:, ci, c0:a_end],
                            start=(kc == 0), stop=(kc == kmax),
                            skip_group_check=True)

                def tail_pieces(b, j, pi):
                    """Yield tail work as small callables so they can be
                    spread through the chunk stream (emitted right after
                    a later chunk's mask+exp, keeping the DVE FIFO from
                    blocking ACT)."""
                    y2 = y2s.pop((b, j, pi))
                    if "no_tail" in ablate:
                        return
                    state = {}

                    def p_recompute():
                        q32 = ps_pool.tile([P, 2, qg], F32, name="q32",
                                           tag="ps")
                        for ci in range(2):
                            h = 2 * pi + ci
                            hp = slice(32 * h, 32 * h + 32)
                            nc.tensor.matmul(
                                q32[:, ci, 0:P], kT32[b][hp, :],
                                qT32[b][hp, :], start=True, stop=True,
                                tile_position=(32 * h, 0))
                        nc.vector.tensor_tensor(
                            q32[:, :, 0:P], q32[:, :, 0:P],
                            cmask[:, None, :].to_broadcast((P, 2, P)),
                            mybir.AluOpType.add)
                        et32 = r_pool.tile([P, 2, P], F32, name="et32",
                                           tag="et32")
                        nc.scalar.activation(
                            et32[:], q32[:, :, 0:P],
                            mybir.ActivationFunctionType.Exp, scale=scale)
                        for ci in range(2):
                            h = 2 * pi + ci
                            nc.tensor.matmul(
                                y2[:, ci, 0:P], vP32[b][:, h, :],
                                et32[:, ci, :],
                                start=True, stop=True,
                                skip_group_check=True)

                    def p_recip():
                        rec = rec_pool.tile([DH, 2, qg], F32, name="rec",
                                            tag="rec")
                        state["rec"] = rec
                        nc.vector.reciprocal(rec[:], y2[DH:2 * DH, :, :])

                    def p_mul(ci):
                        rec = state["rec"]
                        if ci == 0:
                            ysb = ysb_pool.tile([2 * DH, qg], FA,
                                                name="ysb", tag="ysb")
                            state["ysb"] = ysb
                            ysbs.setdefault((b, j), []).append(ysb)
                        ysb = state["ysb"]
                        nc.vector.tensor_mul(ysb[DH * ci:DH * (ci + 1), :],
                                             y2[0:DH, ci, :], rec[:, ci, :])

                    def p_proj(tch):
                        pair = ysbs[(b, j)]
                        t0 = j * qg + tch * P
                        csl = slice(tch * P, (tch + 1) * P)
                        po = ps_pool.tile([P, 2, qg], F32, name="proj",
                                          tag="ps")
                        for pp in range(2):
                            nc.tensor.matmul(
                                po[:, 0, 0:D], pair[pp][:, csl],
                                wpP[pp][:],
                                start=(pp == 0), stop=(pp == 1),
                                skip_group_check=True)
                        ob = out_pool.tile([P, D], F32, name="ob",
                                           tag="ob")
                        nc.vector.tensor_copy(ob[:], po[:, 0, 0:D])
                        nc.sync.dma_start(out_d[b, t0:t0 + P, :], ob[:])
                        if tch == cpq - 1:
                            ysbs.pop((b, j))

                    if recompute and j == 0:
                        yield p_recompute
                    yield p_recip
                    yield lambda: p_mul(0)
                    yield lambda: p_mul(1)
                    if pi == 1:
                        for tch in range(cpq):
                            yield lambda tch=tch: p_proj(tch)

                def emit_A_head(b):
                    emit_A_transpose(b, 0)
                    emit_A_qkv(b, 0)
                    if recompute:
                        emit_A_recompute_qk(b)

                def emit_B_stream(bs, pieces, depth=DEPTH):
                    """Pipelined stream over all (b, j, pi, kc) of the
                    given batches (lockstep-interleaved when several);
                    `pieces` are phase-A callables woven in at group
                    boundaries."""
                    per_b = [[(b, j, pi, kc)
                              for j in range(nqg)
                              for pi in range(2)
                              for kc in range(cpq * (j + 1))]
                             for b in bs]
                    items = [it for tup in zip(*per_b) for it in tup]
                    pend = []     # (item, et) awaiting consumption
                    tail_q = []   # small tail callables, spread out
                    pi_q = list(pieces)

                    def consume_one():
                        (bb, jj, pp, kk), et = pend.pop(0)
                        if kk == 0:
                            # flush tails before the y-slot reallocation
                            while tail_q:
                                tail_q.pop(0)()
                        emit_C(bb, jj, pp, kk, et)
                        if kk == cpq * (jj + 1) - 1:  # group done
                            tail_q.extend(tail_pieces(bb, jj, pp))
                            if pi_q:
                                pi_q.pop(0)()

                    for it in items:
                        et = emit_P(*it)
                        if tail_q:
                            tail_q.pop(0)()
                        pend.append((it, et))
                        if len(pend) > depth:
                            consume_one()
                    while pend:
                        consume_one()
                    while tail_q:
                        tail_q.pop(0)()
                    while pi_q:
                        pi_q.pop(0)()

                if interleave == "batch" and bpc == 2:
                    # all phase A up front (dense PE work warms the HAM
                    # clock gate), then both batches' attention streams
                    # in lockstep: PE always has the other batch's
                    # matmuls while ACT runs this batch's exp.
                    for b in range(bpc):
                        emit_A_head(b)
                        for j in range(1, nqg):
                            emit_A_transpose(b, j)
                            emit_A_qkv(b, j)
                    emit_B_stream([0, 1], [], depth=2 * DEPTH)
                elif interleave and bpc == 2:
                    emit_A_head(0)
                    pieces = [lambda j=j: (emit_A_transpose(0, j),
                                           emit_A_qkv(0, j))
                              for j in range(1, nqg)]
                    pieces += [lambda: emit_A_head(1)]
                    pieces += [lambda j=j: (emit_A_transpose(1, j),
                                            emit_A_qkv(1, j))
                               for j in range(1, nqg)]
                    emit_B_stream([0], pieces)
                    emit_B_stream([1], [])
                else:
                    for b in range(bpc):
                        emit_A_head(b)
                        for j in range(1, nqg):
                            emit_A_transpose(b, j)
                            emit_A_qkv(b, j)
                    for b in range(bpc):
                        emit_B_stream([b], [])
    nc.compile()
    return nc


_NC_CACHE = {}

CONFIG = {"attn_dt": "f16", "recompute": False, "interleave": "batch",
          "quad_bufs": 2, "exp_bufs": 8}


def _get_nc(bpc=BPC, t=T, loop_n=0):
    key = (bpc, t, loop_n)
    if key not in _NC_CACHE:
        _NC_CACHE[key] = build_attention_nc(bpc=bpc, t=t, loop_n=loop_n,
                                            **CONFIG)
    return _NC_CACHE[key]


def _run(x, w_attn, w_proj, **spmd_kwargs):
    x = np.ascontiguousarray(np.asarray(x), dtype=np.float32)
    w_attn = np.ascontiguousarray(np.asarray(w_attn), dtype=np.float32)
    w_proj = np.ascontiguousarray(np.asarray(w_proj), dtype=np.float32)
    assert x.shape == (B, T, D), x.shape

    nc = _get_nc()
    in_maps = [
        {"x": x[c * BPC:(c + 1) * BPC], "w_attn": w_attn, "w_proj": w_proj}
        for c in range(N_CORES)
    ]
    res = run_bass_kernel_spmd(nc, in_maps, list(range(N_CORES)),
                               **spmd_kwargs)
    out = np.concatenate([res.results[c]["out"] for c in range(N_CORES)],
                         axis=0)
    return out.astype(np.float32), res


def kernel(x, w_attn, w_proj):
    out, _ = _run(x, w_attn, w_proj)
    return out


if __name__ == "__main__":
    nc = build_attention_nc()
    print("built ok")
